# revision 9
# baseline (speedup 1.0000x reference)
"""ConcatCritic pairwise-MLP scores on 8 Trainium2 NeuronCores.

scores[i, j] = MLP(concat(x[j], y[i])),  MLP: 256 -> 512 (relu) -> 512 (relu) -> 1

Sharding: data-parallel over y rows (i). Each core holds full x and a
64-row y shard and computes a [64, 512] score block; weights replicated.

Per-core dataflow (everything SBUF-resident, features on partitions):
  setup:  hxT[h, j] = W1x @ x.T      (4 matmuls, [128,128]x[128,512])
          byT[h, i] = W1y @ y.T + b1 (4 matmuls + ACT bias-add)
  per i:  h1T[h, j] = relu(hxT + byT[:, i])        4 DVE tensor_scalar
          z2 [g, j] = W2 @ h1T   (16 accumulating matmuls, fp32r)
          u  [g, j] = relu(z2 + b2)                4 ACT from PSUM
          s  [1, j] = W3 @ u      (4 accumulating matmuls, fp32r)
          DVE copy s -> SBUF, DMA -> out[i]
b3 is added on the host.

All inputs ride in ONE packed [128, 3660] fp32r DRAM tensor: the fused
fp32r matmul lowering has a single sem-wait slot, so matmul operands
must arrive behind a single DMA semaphore.  Packed free-dim layout:
  [0:512)      xT        x.T                 [NX, B]
  [512:1024)   w1xT      W1[:, :NX].T        [NX, H]
  [1024:1536)  w1yT      W1[:, NX:].T        [NY, H]
  [1536:3584)  w2T       [p, k, g] k-major   [P, KT*H]
  [3584:3648)  yT        y_shard.T           [NY, SH]
  [3648:3652)  w3T       [p, m]              [P, MT]
  [3652:3656)  b1c       [p, k]              [P, KT]
  [3656:3660)  b2c       [p, m]              [P, MT]
"""

import numpy as np

B, NX, NY, H = 512, 128, 128, 512
N_CORES = 8
SH = B // N_CORES  # y rows per core
P = 128
KT = H // P  # 4 h-tiles (contraction of layer 2)
MT = H // P  # 4 g-tiles (output of layer 2)

OFF_XT = 0
OFF_W1X = OFF_XT + B
OFF_W1Y = OFF_W1X + H
OFF_W2 = OFF_W1Y + H


def _offsets(sh):
    off_yt = OFF_W2 + KT * H
    off_w3 = off_yt + sh
    off_b1 = off_w3 + MT
    off_b2 = off_b1 + KT
    return off_yt, off_w3, off_b1, off_b2, off_b2 + MT

_CACHE = {}


def build_bass(sh=SH):
    import concourse.mybir as mybir
    import concourse.tile as tile
    from concourse import bacc

    F32 = mybir.dt.float32
    F32R = mybir.dt.float32r
    ALU = mybir.AluOpType
    ACTF = mybir.ActivationFunctionType

    off_yt, off_w3, off_b1, off_b2, total = _offsets(sh)

    nc = bacc.Bacc(name="concat_critic")

    pk_d = nc.dram_tensor("packed", [P, total], F32R, kind="ExternalInput")
    out_d = nc.dram_tensor("out", [sh, B], F32, kind="ExternalOutput")

    with tile.TileContext(nc) as tc:
        with (
            tc.tile_pool(name="const", bufs=1) as cpool,
            tc.tile_pool(name="work", bufs=3) as wpool,
            tc.tile_pool(name="zpsum", bufs=3, space="PSUM") as zpool,
            tc.tile_pool(name="spsum", bufs=2, space="PSUM") as spool,
        ):
            pk = cpool.tile([P, total], F32R)
            nc.sync.dma_start(pk[:], pk_d[:])
            xT = pk[:, OFF_XT:OFF_XT + B]
            w1xT = pk[:, OFF_W1X:OFF_W1X + H]
            w1yT = pk[:, OFF_W1Y:OFF_W1Y + H]
            yT = pk[:, off_yt:off_yt + sh]

            def w2T(k, m):  # lhsT tile [P, P] for (k, m)
                o = OFF_W2 + k * H + m * P
                return pk[:, o:o + P]

            def w3T(m):
                return pk[:, off_w3 + m:off_w3 + m + 1]

            def b1c(k):
                return pk[:, off_b1 + k:off_b1 + k + 1].bitcast(F32)

            def b2c(m):
                return pk[:, off_b2 + m:off_b2 + m + 1].bitcast(F32)

            # --- setup: hxT [h, j] and byT [h, i] = hyT + b1 ---
            hxT = cpool.tile([P, KT, B], F32)
            byT = cpool.tile([P, KT, sh], F32)
            with tc.tile_pool(name="setup_psum", bufs=1, space="PSUM") as supool:
                for m in range(KT):
                    ph = supool.tile([P, B], F32, tag="ph", name="ph")
                    nc.tensor.matmul(
                        ph[:], w1xT[:, m * P:(m + 1) * P], xT,
                        start=True, stop=True,
                    )
                    nc.scalar.copy(hxT[:, m, :], ph[:])
                    py_ = supool.tile([P, sh], F32, tag="py", name="py")
                    nc.tensor.matmul(
                        py_[:], w1yT[:, m * P:(m + 1) * P], yT,
                        start=True, stop=True,
                    )
                    nc.scalar.activation(
                        byT[:, m, :], py_[:], ACTF.Identity,
                        bias=b1c(m), scale=1.0,
                    )

            # --- main loop over local y rows ---
            for i in range(sh):
                h1 = [wpool.tile([P, B], mybir.dt.float32r, tag=f"h1_{k}", name=f"h1_{k}")
                      for k in range(KT)]
                for k in range(KT):
                    nc.vector.tensor_scalar(
                        h1[k][:], hxT[:, k, :], byT[:, k, i:i + 1], 0.0,
                        ALU.add, ALU.max,
                    )
                u = [wpool.tile([P, B], mybir.dt.float32r, tag=f"u_{m}", name=f"u_{m}")
                     for m in range(MT)]
                for m in range(MT):
                    zp = zpool.tile([P, B], F32, tag="z2", name="zp")
                    for k in range(KT):
                        nc.tensor.matmul(
                            zp[:], w2T(k, m), h1[k][:],
                            start=(k == 0), stop=(k == KT - 1),
                        )
                    nc.scalar.activation(
                        u[m][:], zp[:], ACTF.Relu,
                        bias=b2c(m), scale=1.0,
                    )
                sp = spool.tile([1, B], F32, tag="sc", name="sp")
                for m in range(MT):
                    nc.tensor.matmul(
                        sp[:], w3T(m), u[m][:],
                        start=(m == 0), stop=(m == MT - 1),
                    )
                sc_sb = wpool.tile([1, B], F32, tag="sc_sb", name="sc_sb")
                nc.vector.tensor_copy(sc_sb[:], sp[:])
                nc.sync.dma_start(out_d[i:i + 1, :], sc_sb[:])

    nc.finalize()
    return nc


def pack_inputs(x, y, W1, b1, W2, b2, W3, sh=SH):
    """Host-side packing into the single [P, total] input tensor per core."""
    f32 = lambda a: np.asarray(a, dtype=np.float32)
    x, y, W1, b1, W2, b2, W3 = map(f32, (x, y, W1, b1, W2, b2, W3))
    n_cores = y.shape[0] // sh
    xT = x.T                                           # [NX, B]
    w1xT = W1[:, :NX].T                                # [NX, H]
    w1yT = W1[:, NX:].T                                # [NY, H]
    w2r = W2.T.reshape(KT, P, H).transpose(1, 0, 2).reshape(P, KT * H)
    w3T = W3.reshape(MT, P).T                          # [P, MT]
    b1c = b1.reshape(KT, P).T                          # [P, KT]
    b2c = b2.reshape(MT, P).T                          # [P, MT]
    common = np.concatenate([xT, w1xT, w1yT, w2r], axis=1)
    tail = np.concatenate([w3T, b1c, b2c], axis=1)
    maps = []
    for c in range(n_cores):
        yTs = y[c * sh:(c + 1) * sh, :].T              # [NY, sh]
        pk = np.ascontiguousarray(
            np.concatenate([common, yTs, tail], axis=1), dtype=np.float32)
        maps.append({"packed": pk})
    return maps


def run_spmd(in_maps, trace=False, **kw):
    from concourse.bass_utils import run_bass_kernel_spmd

    if "nc" not in _CACHE:
        _CACHE["nc"] = build_bass()
    return run_bass_kernel_spmd(
        _CACHE["nc"], in_maps, core_ids=list(range(N_CORES)), trace=trace, **kw
    )


def kernel(x, y, W1, b1, W2, b2, W3, b3):
    in_maps = pack_inputs(x, y, W1, b1, W2, b2, W3)
    res = run_spmd(in_maps)
    scores = np.concatenate([r["out"] for r in res.results], axis=0)
    scores = (scores + np.float32(np.asarray(b3).reshape(-1)[0])).astype(np.float32)
    return (scores, 0.0)


# revision 10
# speedup vs baseline: 2060.2915x; 2060.2915x over previous
"""ConcatCritic pairwise-MLP scores on 8 Trainium2 NeuronCores.

scores[i, j] = MLP(concat(x[j], y[i])),  MLP: 256 -> 512 (relu) -> 512 (relu) -> 1

Sharding: data-parallel over y rows (i). Each core holds full x and a
64-row y shard and computes a [64, 512] score block; weights replicated.

Per-core dataflow (everything SBUF-resident, features on partitions):
  setup:  hxT[h, j] = W1x @ x.T      (4 matmuls, [128,128]x[128,512])
          byT[h, i] = W1y @ y.T + b1 (4 matmuls + ACT bias-add)
  per i:  h1T[h, j] = relu(hxT + byT[:, i])        4 DVE tensor_scalar
          z2 [g, j] = W2 @ h1T   (16 accumulating matmuls, fp32r)
          u  [g, j] = relu(z2 + b2)                4 ACT from PSUM
          s  [1, j] = W3 @ u      (4 accumulating matmuls, fp32r)
          DVE copy s -> SBUF, DMA -> out[i]
b3 is added on the host.

All inputs ride in ONE packed [128, 3660] fp32r DRAM tensor: the fused
fp32r matmul lowering has a single sem-wait slot, so matmul operands
must arrive behind a single DMA semaphore.  Packed free-dim layout:
  [0:512)      xT        x.T                 [NX, B]
  [512:1024)   w1xT      W1[:, :NX].T        [NX, H]
  [1024:1536)  w1yT      W1[:, NX:].T        [NY, H]
  [1536:3584)  w2T       [p, k, g] k-major   [P, KT*H]
  [3584:3648)  yT        y_shard.T           [NY, SH]
  [3648:3652)  w3T       [p, m]              [P, MT]
  [3652:3656)  b1c       [p, k]              [P, KT]
  [3656:3660)  b2c       [p, m]              [P, MT]
"""

import numpy as np

B, NX, NY, H = 512, 128, 128, 512
N_CORES = 8
SH = B // N_CORES  # y rows per core
P = 128
KT = H // P  # 4 h-tiles (contraction of layer 2)
MT = H // P  # 4 g-tiles (output of layer 2)

OFF_XT = 0
OFF_W1X = OFF_XT + B
OFF_W1Y = OFF_W1X + H
OFF_W2 = OFF_W1Y + H


def _offsets(sh):
    off_yt = OFF_W2 + KT * H
    off_w3 = off_yt + sh
    off_b1 = off_w3 + MT
    off_b2 = off_b1 + KT
    return off_yt, off_w3, off_b1, off_b2, off_b2 + MT

_CACHE = {}


def build_bass(sh=SH):
    import concourse.mybir as mybir
    import concourse.tile as tile
    from concourse import bacc

    F32 = mybir.dt.float32
    F32R = mybir.dt.float32r
    ALU = mybir.AluOpType
    ACTF = mybir.ActivationFunctionType

    off_yt, off_w3, off_b1, off_b2, total = _offsets(sh)

    nc = bacc.Bacc(name="concat_critic")

    pk_d = nc.dram_tensor("packed", [P, total], F32R, kind="ExternalInput")
    out_d = nc.dram_tensor("out", [sh, B], F32, kind="ExternalOutput")

    with tile.TileContext(nc) as tc:
        with (
            tc.tile_pool(name="const", bufs=1) as cpool,
            tc.tile_pool(name="work", bufs=3) as wpool,
            tc.tile_pool(name="zpsum", bufs=3, space="PSUM") as zpool,
            tc.tile_pool(name="spsum", bufs=2, space="PSUM") as spool,
        ):
            pk = cpool.tile([P, total], F32R)
            nc.sync.dma_start(pk[:], pk_d[:])
            xT = pk[:, OFF_XT:OFF_XT + B]
            w1xT = pk[:, OFF_W1X:OFF_W1X + H]
            w1yT = pk[:, OFF_W1Y:OFF_W1Y + H]
            yT = pk[:, off_yt:off_yt + sh]

            def w2T(k, m):  # lhsT tile [P, P] for (k, m)
                o = OFF_W2 + k * H + m * P
                return pk[:, o:o + P]

            def w3T(m):
                return pk[:, off_w3 + m:off_w3 + m + 1]

            def b1c(k):
                return pk[:, off_b1 + k:off_b1 + k + 1].bitcast(F32)

            def b2c(m):
                return pk[:, off_b2 + m:off_b2 + m + 1].bitcast(F32)

            # --- setup: hxT [h, j] and byT [h, i] = hyT + b1 ---
            hxT = cpool.tile([P, KT, B], F32)
            byT = cpool.tile([P, KT, sh], F32)
            with tc.tile_pool(name="setup_psum", bufs=1, space="PSUM") as supool:
                for m in range(KT):
                    ph = supool.tile([P, B], F32, tag="ph", name="ph")
                    nc.tensor.matmul(
                        ph[:], w1xT[:, m * P:(m + 1) * P], xT,
                        start=True, stop=True,
                    )
                    nc.scalar.copy(hxT[:, m, :], ph[:])
                    py_ = supool.tile([P, sh], F32, tag="py", name="py")
                    nc.tensor.matmul(
                        py_[:], w1yT[:, m * P:(m + 1) * P], yT,
                        start=True, stop=True,
                    )
                    nc.scalar.activation(
                        byT[:, m, :], py_[:], ACTF.Identity,
                        bias=b1c(m), scale=1.0,
                    )

            # --- main loop over local y rows ---
            # Layer 3 for row i is emitted one iteration late (after layer 2
            # of row i+1 has been queued) so its matmuls never stall the PE
            # waiting on the freshly-computed relu outputs.
            pend = None  # (i, u_tiles) awaiting layer 3

            def emit_l3(i, u):
                sp = spool.tile([1, B], F32, tag="sc", name="sp")
                for m in range(MT):
                    nc.tensor.matmul(
                        sp[:], w3T(m), u[m][:],
                        start=(m == 0), stop=(m == MT - 1),
                    )
                sc_sb = wpool.tile([1, B], F32, tag="sc_sb", name="sc_sb")
                nc.vector.tensor_copy(sc_sb[:], sp[:])
                nc.sync.dma_start(out_d[i:i + 1, :], sc_sb[:])

            for i in range(sh):
                h1 = [wpool.tile([P, B], mybir.dt.float32r, tag=f"h1_{k}", name=f"h1_{k}")
                      for k in range(KT)]
                for k in range(KT):
                    nc.vector.tensor_scalar(
                        h1[k][:], hxT[:, k, :], byT[:, k, i:i + 1], 0.0,
                        ALU.add, ALU.max,
                    )
                u = [wpool.tile([P, B], mybir.dt.float32r, tag=f"u_{m}", name=f"u_{m}")
                     for m in range(MT)]
                for m in range(MT):
                    zp = zpool.tile([P, B], F32, tag="z2", name="zp")
                    for k in range(KT):
                        nc.tensor.matmul(
                            zp[:], w2T(k, m), h1[k][:],
                            start=(k == 0), stop=(k == KT - 1),
                        )
                    nc.scalar.activation(
                        u[m][:], zp[:], ACTF.Relu,
                        bias=b2c(m), scale=1.0,
                    )
                if pend is not None:
                    emit_l3(*pend)
                pend = (i, u)
            emit_l3(*pend)

    nc.finalize()
    return nc


def pack_inputs(x, y, W1, b1, W2, b2, W3, sh=SH):
    """Host-side packing into the single [P, total] input tensor per core."""
    f32 = lambda a: np.asarray(a, dtype=np.float32)
    x, y, W1, b1, W2, b2, W3 = map(f32, (x, y, W1, b1, W2, b2, W3))
    n_cores = y.shape[0] // sh
    xT = x.T                                           # [NX, B]
    w1xT = W1[:, :NX].T                                # [NX, H]
    w1yT = W1[:, NX:].T                                # [NY, H]
    w2r = W2.T.reshape(KT, P, H).transpose(1, 0, 2).reshape(P, KT * H)
    w3T = W3.reshape(MT, P).T                          # [P, MT]
    b1c = b1.reshape(KT, P).T                          # [P, KT]
    b2c = b2.reshape(MT, P).T                          # [P, MT]
    common = np.concatenate([xT, w1xT, w1yT, w2r], axis=1)
    tail = np.concatenate([w3T, b1c, b2c], axis=1)
    maps = []
    for c in range(n_cores):
        yTs = y[c * sh:(c + 1) * sh, :].T              # [NY, sh]
        pk = np.ascontiguousarray(
            np.concatenate([common, yTs, tail], axis=1), dtype=np.float32)
        maps.append({"packed": pk})
    return maps


def run_spmd(in_maps, trace=False, **kw):
    from concourse.bass_utils import run_bass_kernel_spmd

    if "nc" not in _CACHE:
        _CACHE["nc"] = build_bass()
    return run_bass_kernel_spmd(
        _CACHE["nc"], in_maps, core_ids=list(range(N_CORES)), trace=trace, **kw
    )


def kernel(x, y, W1, b1, W2, b2, W3, b3):
    in_maps = pack_inputs(x, y, W1, b1, W2, b2, W3)
    res = run_spmd(in_maps)
    scores = np.concatenate([r["out"] for r in res.results], axis=0)
    scores = (scores + np.float32(np.asarray(b3).reshape(-1)[0])).astype(np.float32)
    return (scores, 0.0)


# revision 11
# speedup vs baseline: 2170.6816x; 1.0536x over previous
"""ConcatCritic pairwise-MLP scores on 8 Trainium2 NeuronCores.

scores[i, j] = MLP(concat(x[j], y[i])),  MLP: 256 -> 512 (relu) -> 512 (relu) -> 1

Sharding: data-parallel over y rows (i). Each core holds full x and a
64-row y shard and computes a [64, 512] score block; weights replicated.

Per-core dataflow (everything SBUF-resident, features on partitions):
  setup:  hxT[h, j] = W1x @ x.T      (4 matmuls, [128,128]x[128,512])
          byT[h, i] = W1y @ y.T + b1 (4 matmuls + ACT bias-add)
  per i:  h1T[h, j] = relu(hxT + byT[:, i])          4 DVE tensor_scalar
          z2 [g, j] = W2 @ h1T     (16 accumulating fp32r matmuls)
          u'_m[g,j] = |w3| * relu(z2 + b2)           4 ACT (scale trick)
          s   [g,j] = sum_m u'_m                     2 GPSIMD + 1 DVE add
          sc  [1,j] = sgn . s (+ corr . u'_c)        2 accumulating matmuls
          ACT copy sc -> SBUF, DMA -> out[i]

Layer-3 trick: |W3| rides for free in the relu's per-partition scale
(relu(|w|z+|w|b) = |w|relu(z+b)), and the hidden dim g is permuted so
each partition's 4 chunk elements share one sign -- the sign goes into
the final matmul's stationary vector.  randn W3 can't always be
4-sign-partitioned exactly, so up to 2 "minority" elements sit in the
top chunks and get a -2 correction matmul against that chunk.
b3 is added on the host.

Inputs ride in TWO fp32r DRAM tensors (packed pk + w2) so setup can
start before the 1MB W2 finishes loading.  pk free-dim layout:
  [0:512)      xT      x.T                    [NX, B]
  [512:1024)   w1xT    W1[:, :NX].T           [NX, H]
  [1024:1536)  w1yT    W1[:, NX:].T           [NY, H]
  [1536:1536+sh) yT    y_shard.T              [NY, sh]
  then per-column: aw3[4] b1c[4] b2s[4] sgn[1] corr[2]
w2 is [P, KT*H]: [p, k, g] with the g-permutation applied.
"""

import numpy as np

B, NX, NY, H = 512, 128, 128, 512
N_CORES = 8
SH = B // N_CORES  # y rows per core
P = 128
KT = H // P  # 4 h-tiles (contraction of layer 2)
MT = H // P  # 4 g-tiles (output of layer 2)

OFF_XT = 0
OFF_W1X = OFF_XT + B
OFF_W1Y = OFF_W1X + H
OFF_YT = OFF_W1Y + H


def _offsets(sh):
    off_aw3 = OFF_YT + sh
    off_b1 = off_aw3 + MT
    off_b2s = off_b1 + KT
    off_sgn = off_b2s + MT
    off_corr = off_sgn + 1
    return off_aw3, off_b1, off_b2s, off_sgn, off_corr, off_corr + 2

_CACHE = {}


def build_bass(sh=SH, n_corr=1):
    import concourse.mybir as mybir
    import concourse.tile as tile
    from concourse import bacc

    F32 = mybir.dt.float32
    F32R = mybir.dt.float32r
    ALU = mybir.AluOpType
    ACTF = mybir.ActivationFunctionType

    off_aw3, off_b1, off_b2s, off_sgn, off_corr, total = _offsets(sh)

    nc = bacc.Bacc(name="concat_critic")

    pk_d = nc.dram_tensor("packed", [P, total], F32R, kind="ExternalInput")
    w2_d = nc.dram_tensor("w2", [P, KT * H], F32R, kind="ExternalInput")
    out_d = nc.dram_tensor("out", [sh, B], F32, kind="ExternalOutput")

    with tile.TileContext(nc) as tc:
        with (
            tc.tile_pool(name="const", bufs=1) as cpool,
            tc.tile_pool(name="work", bufs=3) as wpool,
            tc.tile_pool(name="zpsum", bufs=3, space="PSUM") as zpool,
            tc.tile_pool(name="spsum", bufs=2, space="PSUM") as spool,
        ):
            pk = cpool.tile([P, total], F32R)
            nc.sync.dma_start(pk[:], pk_d[:])
            w2 = cpool.tile([P, KT * H], F32R)
            nc.sync.dma_start(w2[:], w2_d[:])
            xT = pk[:, OFF_XT:OFF_XT + B]
            w1xT = pk[:, OFF_W1X:OFF_W1X + H]
            w1yT = pk[:, OFF_W1Y:OFF_W1Y + H]
            yT = pk[:, OFF_YT:OFF_YT + sh]

            def w2T(k, m):  # lhsT tile [P, P] for (k, m)
                o = k * H + m * P
                return w2[:, o:o + P]

            def col(off, j, cast=None):
                c = pk[:, off + j:off + j + 1]
                return c.bitcast(F32) if cast else c

            # --- setup: hxT [h, j] and byT [h, i] = hyT + b1 ---
            hxT = cpool.tile([P, KT, B], F32)
            byT = cpool.tile([P, KT, sh], F32)
            with tc.tile_pool(name="setup_psum", bufs=1, space="PSUM") as supool:
                for m in range(KT):
                    ph = supool.tile([P, B], F32, tag="ph", name="ph")
                    nc.tensor.matmul(
                        ph[:], w1xT[:, m * P:(m + 1) * P], xT,
                        start=True, stop=True,
                    )
                    nc.scalar.copy(hxT[:, m, :], ph[:])
                    py_ = supool.tile([P, sh], F32, tag="py", name="py")
                    nc.tensor.matmul(
                        py_[:], w1yT[:, m * P:(m + 1) * P], yT,
                        start=True, stop=True,
                    )
                    nc.scalar.activation(
                        byT[:, m, :], py_[:], ACTF.Identity,
                        bias=col(off_b1, m, cast=True), scale=1.0,
                    )

            # --- main loop over local y rows ---
            # Layer 3 for row i is emitted one iteration late (after layer 2
            # of row i+1 has been queued) so its matmuls never stall the PE
            # waiting on fresh relu outputs.
            pend = None

            def emit_l3(i, u):
                t0 = wpool.tile([P, B], F32, tag="t0", name="t0")
                t1 = wpool.tile([P, B], F32, tag="t1", name="t1")
                nc.gpsimd.tensor_tensor(
                    t0[:], u[0][:].bitcast(F32), u[1][:].bitcast(F32), ALU.add)
                nc.gpsimd.tensor_tensor(
                    t1[:], u[2][:].bitcast(F32), u[3][:].bitcast(F32), ALU.add)
                s = wpool.tile([P, B], F32R, tag="s", name="s")
                nc.vector.tensor_tensor(s[:], t0[:], t1[:], ALU.add)
                sp = spool.tile([1, B], F32, tag="sc", name="sp")
                nc.tensor.matmul(
                    sp[:], col(off_sgn, 0), s[:],
                    start=True, stop=(n_corr == 0),
                )
                for c in range(n_corr):
                    nc.tensor.matmul(
                        sp[:], col(off_corr, c), u[MT - 1 - c][:],
                        start=False, stop=(c == n_corr - 1),
                    )
                sc_sb = wpool.tile([1, B], F32, tag="sc_sb", name="sc_sb")
                nc.scalar.copy(sc_sb[:], sp[:])
                nc.sync.dma_start(out_d[i:i + 1, :], sc_sb[:])

            for i in range(sh):
                h1 = [wpool.tile([P, B], F32R, tag=f"h1_{k}", name=f"h1_{k}")
                      for k in range(KT)]
                for k in range(KT):
                    nc.vector.tensor_scalar(
                        h1[k][:], hxT[:, k, :], byT[:, k, i:i + 1], 0.0,
                        ALU.add, ALU.max,
                    )
                u = [wpool.tile([P, B], F32R, tag=f"u_{m}", name=f"u_{m}")
                     for m in range(MT)]
                for m in range(MT):
                    zp = zpool.tile([P, B], F32, tag="z2", name="zp")
                    for k in range(KT):
                        nc.tensor.matmul(
                            zp[:], w2T(k, m), h1[k][:],
                            start=(k == 0), stop=(k == KT - 1),
                        )
                    nc.scalar.activation(
                        u[m][:], zp[:], ACTF.Relu,
                        bias=col(off_b2s, m, cast=True),
                        scale=col(off_aw3, m, cast=True),
                    )
                if pend is not None:
                    emit_l3(*pend)
                pend = (i, u)
            emit_l3(*pend)

    nc.finalize()
    return nc


def _sign_partition(w3):
    """Permute g (the hidden dim of layer 2's output) so each partition's 4
    chunk elements share a sign, except <=1 mixed quad whose minority
    elements land in the top chunks.  Returns (perm, sgn[P], n_corr,
    corr[2][P]) with perm[new_slot] = old_g, new_slot = chunk*P + p."""
    w3 = np.asarray(w3).reshape(-1)
    pos = sorted(np.nonzero(w3 > 0)[0].tolist())
    neg = sorted(np.nonzero(w3 <= 0)[0].tolist())
    npos_q, rpos = divmod(len(pos), 4)
    slots = np.empty((MT, P), dtype=np.int64)  # [chunk, p] -> old g
    sgn = np.empty(P, dtype=np.float32)
    corr = np.zeros((2, P), dtype=np.float32)
    p = 0
    for q in range(npos_q):  # all-positive quads
        for c in range(MT):
            slots[c, p] = pos.pop()
        sgn[p] = 1.0
        p += 1
    n_corr = 0
    if rpos:  # one mixed quad: leftovers of both signs
        rest = [pos.pop() for _ in range(len(pos))] + \
               [neg.pop() for _ in range(4 - rpos)]
        # majority sign of the quad
        maj = 1.0 if rpos >= 2 else -1.0
        sgn[p] = maj
        # minority elements go in the TOP chunks (3, then 2)
        minority = rest[rpos:] if maj > 0 else rest[:rpos]
        majority = rest[:rpos] if maj > 0 else rest[rpos:]
        order = majority + minority  # chunks 0.. hold majority first
        for c in range(MT):
            slots[c, p] = order[c]
        n_corr = len(minority)
        assert n_corr <= 2
        for idx in range(n_corr):
            corr[idx, p] = -2.0 * maj  # flip the minority contribution
        p += 1
    while p < P:  # all-negative quads
        for c in range(MT):
            slots[c, p] = neg.pop()
        sgn[p] = -1.0
        p += 1
    assert not pos and not neg
    perm = slots.reshape(-1)  # new_slot = c*P + p -> old g
    return perm, sgn, n_corr, corr


def pack_inputs(x, y, W1, b1, W2, b2, W3, sh=SH):
    """Host-side packing into the per-core input tensors."""
    f32 = lambda a: np.asarray(a, dtype=np.float32)
    x, y, W1, b1, W2, b2, W3 = map(f32, (x, y, W1, b1, W2, b2, W3))
    n_cores = y.shape[0] // sh

    w3 = W3.reshape(-1)
    perm, sgn, n_corr, corr = _sign_partition(w3)
    # permuted layer-2 output dim: chunk-major [c*P + p]
    W2p = W2[perm, :]                                  # [H, H] rows permuted
    b2p = b2[perm]
    aw3 = np.abs(w3[perm])
    b2s = aw3 * b2p

    xT = x.T                                           # [NX, B]
    w1xT = W1[:, :NX].T                                # [NX, H]
    w1yT = W1[:, NX:].T                                # [NY, H]
    # w2 tile layout [p, k, g']: contraction dim h (un-permuted) on
    # partitions, permuted g' on free
    w2r = W2p.T.reshape(KT, P, H).transpose(1, 0, 2).reshape(P, KT * H)
    aw3c = aw3.reshape(MT, P).T                        # [P, MT]
    b1c = b1.reshape(KT, P).T                          # [P, KT]
    b2sc = b2s.reshape(MT, P).T                        # [P, MT]
    tail = np.concatenate(
        [aw3c, b1c, b2sc, sgn[:, None], corr.T], axis=1)
    common = np.concatenate([xT, w1xT, w1yT], axis=1)
    w2pk = np.ascontiguousarray(w2r, dtype=np.float32)
    maps = []
    for c in range(n_cores):
        yTs = y[c * sh:(c + 1) * sh, :].T              # [NY, sh]
        pk = np.ascontiguousarray(
            np.concatenate([common, yTs, tail], axis=1), dtype=np.float32)
        maps.append({"packed": pk, "w2": w2pk})
    return maps, n_corr


def run_spmd(in_maps, n_corr=1, trace=False, **kw):
    from concourse.bass_utils import run_bass_kernel_spmd

    key = ("nc", n_corr)
    if key not in _CACHE:
        _CACHE[key] = build_bass(n_corr=n_corr)
    return run_bass_kernel_spmd(
        _CACHE[key], in_maps, core_ids=list(range(N_CORES)), trace=trace, **kw
    )


def kernel(x, y, W1, b1, W2, b2, W3, b3):
    in_maps, n_corr = pack_inputs(x, y, W1, b1, W2, b2, W3)
    res = run_spmd(in_maps, n_corr=n_corr)
    scores = np.concatenate([r["out"] for r in res.results], axis=0)
    scores = (scores + np.float32(np.asarray(b3).reshape(-1)[0])).astype(np.float32)
    return (scores, 0.0)
